# revision 13
# baseline (speedup 1.0000x reference)
"""Trainium2 Bass kernel for nn_Block_41077067219413.

Reference computation (B=2048, D=dim_in=4096, J=dim_out=4096):
    xf = x.astype(f32)                 # (B, D) in {0,1}
    mf = masks.astype(f32)             # (D, J) in {0,1}
    sums = xf @ mf + (1-xf) @ (1-mf)   # XNOR popcount over D
    out  = sums > thresholds[None, :]  # (B, J) bool

Identity: with x' = 2x-1 and m' = 2m-1 (both in {-1,+1}),
    A'[j,b] = sum_k m'[k,j] * x'[b,k] = 2*sums - D
    out     = A' > 2*th - D

Both operands are host-encoded as fp8e4 bytes (+1 = 0x38, -1 = 0xB8) and
host-tiled so every DMA row is 4096 contiguous bytes (per-descriptor
issue overhead on the DGE queues is the input-bandwidth limit).  The
device runs one fp8 DoubleRow GEMM per core with no transposes, rowsum,
or threshold folding.  Work is sharded 2 (batch) x 4 (dim_out): each
core computes out_shard [1024 j, 1024 b] = m'^T @ x' with j on PSUM
partitions, so the threshold compare is a per-partition-scalar is_gt
(vector engine) or Sign activation (scalar engine), split across both.

Schedule: warm-up matmuls on a zeroed tile un-throttle the PE clock
(HAM) while the first tiles stream in, then 4 passes of (j-half,
b-half), 64 DR matmuls each, alternating two 4-bank PSUM sets so
epilogues overlap the next pass.  Input DMAs are tiered (small first
tiles for a fast ramp) and placed on the two hardware-DGE queues
(scalar/sync) for the critical path; the slower software-DGE gpsimd
queue only carries late tiles.  The DMA count stays low enough that
DMA-semaphore reuse never targets an in-flight transfer.
"""

import numpy as np

B, D, J = 2048, 4096, 4096
NCORES = 8
BS, JS = 2, 4             # batch-shards x j-shards
BL = B // BS              # 1024 batch rows per core
JL = J // JS              # 1024 output cols per core
P = 128
KP = D // 256             # 16 k-pair steps (256 contraction each)
NQQ = KP // 4             # 4 dram row-blocks (4 k-pairs = 4096B rows)
JT = JL // P              # 8 j-tiles of 128
BC = 512                  # batch free-dim chunk (one PSUM bank)
NBC = BL // BC            # 2 batch chunks
JH = JT // 2              # 4 j-tiles per pass
NWARM = 44

_cache = {}


def _build():
    import concourse.bacc as bacc
    import concourse.mybir as mybir
    import concourse.tile as tile

    dt = mybir.dt
    f8 = dt.float8e4
    f32 = dt.float32
    ALU = mybir.AluOpType
    AF = mybir.ActivationFunctionType
    DR = mybir.MatmulPerfMode.DoubleRow

    nc = bacc.Bacc("TRN2", target_bir_lowering=False, debug=False,
                   num_devices=NCORES)

    # host-tiled fp8 bytes; row r = (chunk*NQQ + kqq)*128 + ki holds 4096
    # contiguous bytes [kql | kpp | ko | c] for that partition
    x_d = nc.dram_tensor("xp", [NBC * NQQ * P, 4096], dt.uint8,
                         kind="ExternalInput")
    m_d = nc.dram_tensor("mp", [2 * NQQ * P, 4096], dt.uint8,
                         kind="ExternalInput")
    c_d = nc.dram_tensor("cth", [P, JT], f32, kind="ExternalInput")
    # -(c+1) for the Sign-based epilogue on the Activation engine
    cn_d = nc.dram_tensor("cng", [P, JT], f32, kind="ExternalInput")
    o_d = nc.dram_tensor("out", [JL, BL], dt.uint8, kind="ExternalOutput")

    with tile.TileContext(nc) as tc:
        with (
            tc.tile_pool(name="const", bufs=1) as constp,
            tc.tile_pool(name="mk", bufs=1) as mkp,
            tc.tile_pool(name="xk", bufs=1) as xkp,
            tc.tile_pool(name="ob", bufs=1) as obp,
        ):
            # warm-up source: small zeroed tile, no DMA dependency
            wz = constp.tile([P, 2, 64], dt.uint8)
            nc.vector.memset(wz[:], 0)

            # --- input tiles -------------------------------------------
            # chunk 0 (jh0 masks / bc0 x): kp-split tiles for kp0/kp1,
            # one kq tile for kp2/3, then 4-kp (512KB) tiles; chunk 1:
            # four 4-kp tiles.
            def hk_src(t_d, kp):
                return t_d[0:P, kp * 1024:(kp + 1) * 1024].rearrange(
                    "p (ko c) -> p ko c", ko=2)

            def kq1_src(t_d):
                return t_d[0:P, 2048:4096].rearrange(
                    "p (kpp ko c) -> p kpp ko c", kpp=2, ko=2)

            def kqq_src(t_d, ch, kqq):
                r0 = (ch * NQQ + kqq) * P
                return t_d[r0:r0 + P, :].rearrange(
                    "p (kql kpp ko c) -> p kql kpp ko c", kql=2, kpp=2, ko=2)

            hk = {}   # (op, kp) -> [P, 2, BC]        op 0 = mask, 1 = x
            kq1 = {}  # op -> [P, 2, 2, BC]
            kqq = {}  # (op, ch, kqq) -> [P, 2, 2, 2, BC]
            for op, pool in ((0, mkp), (1, xkp)):
                for kp in range(2):
                    hk[(op, kp)] = pool.tile([P, 2, BC], dt.uint8,
                                             name=f"hk{op}_{kp}")
                kq1[op] = pool.tile([P, 2, 2, BC], dt.uint8,
                                    name=f"kq1_{op}")
                for ch in range(2):
                    for q in range(NQQ):
                        if ch == 0 and q == 0:
                            continue
                        kqq[(op, ch, q)] = pool.tile(
                            [P, 2, 2, 2, BC], dt.uint8,
                            name=f"kqq{op}_{ch}_{q}")

            # (dst, src, queue): criticals on the two HWDGE queues
            # (scalar 's', sync 'y'); late tiles on software-DGE gpsimd 'g'
            plan = [
                (hk[(0, 0)], hk_src(m_d, 0), 's'),
                (hk[(1, 0)], hk_src(x_d, 0), 'y'),
                (hk[(0, 1)], hk_src(m_d, 1), 's'),
                (hk[(1, 1)], hk_src(x_d, 1), 'y'),
                (kq1[0], kq1_src(m_d), 's'),
                (kq1[1], kq1_src(x_d), 'y'),
                (kqq[(0, 0, 1)], kqq_src(m_d, 0, 1), 's'),
                (kqq[(1, 0, 1)], kqq_src(x_d, 0, 1), 'y'),
                ('consts', None, None),
                (kqq[(0, 0, 2)], kqq_src(m_d, 0, 2), 'g'),
                (kqq[(1, 0, 2)], kqq_src(x_d, 0, 2), 's'),
                (kqq[(0, 0, 3)], kqq_src(m_d, 0, 3), 'y'),
                (kqq[(1, 0, 3)], kqq_src(x_d, 0, 3), 'g'),
                (kqq[(1, 1, 0)], kqq_src(x_d, 1, 0), 's'),
                (kqq[(1, 1, 1)], kqq_src(x_d, 1, 1), 'y'),
                (kqq[(1, 1, 2)], kqq_src(x_d, 1, 2), 'g'),
                (kqq[(1, 1, 3)], kqq_src(x_d, 1, 3), 's'),
                (kqq[(0, 1, 0)], kqq_src(m_d, 1, 0), 'y'),
                (kqq[(0, 1, 1)], kqq_src(m_d, 1, 1), 'g'),
                (kqq[(0, 1, 2)], kqq_src(m_d, 1, 2), 's'),
                (kqq[(0, 1, 3)], kqq_src(m_d, 1, 3), 'y'),
            ]
            qmap = {'g': nc.gpsimd, 's': nc.scalar, 'y': nc.sync}
            cth = constp.tile([P, JT], f32)
            cng = constp.tile([P, JT], f32)
            for item in plan:
                if item[0] == 'consts':
                    nc.sync.dma_start(cth[:], c_d[:])
                    nc.sync.dma_start(cng[:], cn_d[:])
                    continue
                dst, src, q = item
                qmap[q].dma_start(dst[:], src)

            def mm_lhsT(jh, kp, j4):
                jsl = slice(j4 * P, (j4 + 1) * P)
                if jh == 0 and kp < 2:
                    return hk[(0, kp)][:, :, jsl]
                if jh == 0 and kp < 4:
                    return kq1[0][:, kp % 2, :, jsl]
                return kqq[(0, jh, kp // 4)][:, (kp % 4) // 2, kp % 2, :, jsl]

            def mm_rhs(bc, kp):
                if bc == 0 and kp < 2:
                    return hk[(1, kp)][:]
                if bc == 0 and kp < 4:
                    return kq1[1][:, kp % 2, :, :]
                return kqq[(1, bc, kp // 4)][:, (kp % 4) // 2, kp % 2, :, :]

            # fused output tiles: one per jt-pair [P, 2, BL]
            obs = [obp.tile([P, 2, BL], dt.uint8, name=f"ob{jp}")
                   for jp in range(JT // 2)]

            with tc.tile_pool(name="psacc", bufs=1, space="PSUM") as psacc:
                # PE warm-up: un-throttle HAM while first tiles land
                wps = psacc.tile([P, BC], f32, name="warm", tag="acc1_0")
                for i in range(NWARM):
                    nc.tensor.matmul(
                        wps[0:32, 0:64], wz[:, :, 0:32].bitcast(f8),
                        wz[:].bitcast(f8),
                        start=(i == 0), stop=(i == NWARM - 1), perf_mode=DR)

                passes = [(0, 0), (0, 1), (1, 0), (1, 1)]
                oqs = [nc.sync, nc.scalar]
                for pi, (jh, bc) in enumerate(passes):
                    alt = pi % 2
                    ps = [psacc.tile([P, BC], f32, name=f"acc{pi}_{j4}",
                                     tag=f"acc{alt}_{j4}")
                          for j4 in range(JH)]
                    for kp in range(KP):
                        for j4 in range(JH):
                            nc.tensor.matmul(
                                ps[j4][:],
                                mm_lhsT(jh, kp, j4).bitcast(f8),
                                mm_rhs(bc, kp).bitcast(f8),
                                start=(kp == 0), stop=(kp == KP - 1),
                                perf_mode=DR)
                    for j4 in range(JH):
                        jt = jh * JH + j4
                        osl = obs[jt // 2][:, jt % 2,
                                           bc * BC:(bc + 1) * BC]
                        if j4 % 2:
                            # A' and c are both even, so A'-(c+1) is odd:
                            # Sign never sees 0 and the strict compare is
                            # exact; the uint8 cast saturates -1 to 0.
                            nc.scalar.activation(
                                osl, ps[j4][:], AF.Sign,
                                bias=cng[:, jt:jt + 1], scale=1.0)
                        else:
                            nc.vector.tensor_scalar(
                                osl, ps[j4][:], cth[:, jt:jt + 1], None,
                                op0=ALU.is_gt)
                        if bc == NBC - 1 and j4 % 2:
                            jp = jt // 2
                            dst = o_d[jp * 2 * P:(jp + 1) * 2 * P,
                                      :].rearrange("(j2 p) b -> p j2 b", p=P)
                            oqs[jp % 2].dma_start(dst, obs[jp][:])

    nc.compile()
    return nc


def _get_nc():
    if "nc" not in _cache:
        _cache["nc"] = _build()
    return _cache["nc"]


def _encode_pm1(a01):
    """{0,1} array -> fp8e4 bytes for {-1,+1} (0xB8 / 0x38)."""
    return np.where(a01, np.uint8(0x38), np.uint8(0xB8))


def _tile_k_major(shard):
    """[4096, 1024] byte array (k-major) -> [2*NQQ*128, 4096]: row
    (chunk*NQQ + kqq)*128 + ki = 4096 contiguous bytes covering four
    k-pairs' ko-interleaved halves of one 512-column chunk."""
    t = shard.reshape(NQQ, 2, 2, 2, P, 2, BC)  # [kqq,kql,kpp,ko,ki,ch,c]
    t = t.transpose(5, 0, 4, 1, 2, 3, 6)       # [ch,kqq,ki,kql,kpp,ko,c]
    return np.ascontiguousarray(t.reshape(2 * NQQ * P, 4096))


def run(x, masks, thresholds, trace=False):
    """Run the SPMD kernel on 8 cores. Returns (out_bool, BassKernelResults)."""
    from concourse.bass_utils import run_bass_kernel_spmd

    nc = _get_nc()
    xT8 = np.ascontiguousarray(_encode_pm1(x.T != 0))          # [D, B]
    m8 = _encode_pm1(np.asarray(masks))                        # [D, J]
    cth = (2.0 * thresholds.astype(np.float32) - float(D))     # [J]
    in_maps = []
    for c in range(NCORES):
        bh, jq = c // JS, c % JS
        in_maps.append({
            "xp": _tile_k_major(xT8[:, bh * BL:(bh + 1) * BL]),
            "mp": _tile_k_major(m8[:, jq * JL:(jq + 1) * JL]),
            "cth": np.ascontiguousarray(
                cth[jq * JL:(jq + 1) * JL].reshape(JT, P).T),
            "cng": np.ascontiguousarray(
                -(cth[jq * JL:(jq + 1) * JL] + 1.0).reshape(JT, P).T),
        })
    res = run_bass_kernel_spmd(nc, in_maps, core_ids=list(range(NCORES)),
                               trace=trace)
    out = np.empty((B, J), dtype=np.uint8)
    for c in range(NCORES):
        bh, jq = c // JS, c % JS
        out[bh * BL:(bh + 1) * BL, jq * JL:(jq + 1) * JL] = \
            res.results[c]["out"].T
    return out.view(np.bool_), res


def kernel(x, masks, thresholds):
    x = np.asarray(x)
    masks = np.asarray(masks)
    thresholds = np.asarray(thresholds)
    out, _ = run(x, masks, thresholds, trace=False)
    return out


# revision 14
# speedup vs baseline: 1.0251x; 1.0251x over previous
"""Trainium2 Bass kernel for nn_Block_41077067219413.

Reference computation (B=2048, D=dim_in=4096, J=dim_out=4096):
    xf = x.astype(f32)                 # (B, D) in {0,1}
    mf = masks.astype(f32)             # (D, J) in {0,1}
    sums = xf @ mf + (1-xf) @ (1-mf)   # XNOR popcount over D
    out  = sums > thresholds[None, :]  # (B, J) bool

Identity: with x' = 2x-1 and m' = 2m-1 (both in {-1,+1}),
    A'[j,b] = sum_k m'[k,j] * x'[b,k] = 2*sums - D
    out     = A' > 2*th - D

Both operands are host-encoded as fp8e4 bytes (+1 = 0x38, -1 = 0xB8) and
host-tiled so every DMA row is 2048 contiguous bytes.  The device runs
one fp8 DoubleRow GEMM per core with no transposes, rowsum, or
threshold folding.  Work is sharded 2 (batch) x 4 (dim_out): each core
computes out_shard [1024 j, 1024 b] = m'^T @ x' with j on PSUM
partitions, so the threshold compare is a per-partition-scalar is_gt
(vector engine) or Sign activation (scalar engine), split across both.

Input DMA queues are rate-weighted (measured: scalar/sync HWDGE ~90
GB/s each, gpsimd SWDGE ~135 GB/s but later start) and tiles are
enqueued in consumption order at 128-256KB granularity so the matmul
stream starts at ~10us and pass 1 runs at DMA speed without long
stalls (which would re-throttle the PE clock).  Warm-up matmuls on a
zeroed tile hold the PE busy (HAM un-throttle) until data lands.
4 passes of (j-half, b-half), 64 DR matmuls each, alternate two 4-bank
PSUM sets so epilogues overlap the next pass's matmuls.
"""

import numpy as np

B, D, J = 2048, 4096, 4096
NCORES = 8
BS, JS = 2, 4             # batch-shards x j-shards
BL = B // BS              # 1024 batch rows per core
JL = J // JS              # 1024 output cols per core
P = 128
KP = D // 256             # 16 k-pair steps (256 contraction each)
KQ = KP // 2              # 8 dram row-blocks (2 k-pairs = 2048B rows)
JT = JL // P              # 8 j-tiles of 128
BC = 512                  # batch free-dim chunk (one PSUM bank)
NBC = BL // BC            # 2 batch chunks
JH = JT // 2              # 4 j-tiles per pass
NWARM = 56

_cache = {}


def _build():
    import concourse.bacc as bacc
    import concourse.mybir as mybir
    import concourse.tile as tile

    dt = mybir.dt
    f8 = dt.float8e4
    f32 = dt.float32
    ALU = mybir.AluOpType
    AF = mybir.ActivationFunctionType
    DR = mybir.MatmulPerfMode.DoubleRow

    nc = bacc.Bacc("TRN2", target_bir_lowering=False, debug=False,
                   num_devices=NCORES)

    # host-tiled fp8 bytes; row r = (chunk*KQ + kq)*128 + ki holds 2048
    # contiguous bytes [kpp=0: ko0 512 | ko1 512 | kpp=1: ko0 | ko1]
    x_d = nc.dram_tensor("xp", [NBC * KQ * P, 2048], dt.uint8,
                         kind="ExternalInput")
    m_d = nc.dram_tensor("mp", [2 * KQ * P, 2048], dt.uint8,
                         kind="ExternalInput")
    c_d = nc.dram_tensor("cth", [P, JT], f32, kind="ExternalInput")
    # -(c+1) for the Sign-based epilogue on the Activation engine
    cn_d = nc.dram_tensor("cng", [P, JT], f32, kind="ExternalInput")
    o_d = nc.dram_tensor("out", [JL, BL], dt.uint8, kind="ExternalOutput")

    with tile.TileContext(nc) as tc:
        with (
            tc.tile_pool(name="const", bufs=1) as constp,
            tc.tile_pool(name="mk", bufs=1) as mkp,
            tc.tile_pool(name="xk", bufs=1) as xkp,
            tc.tile_pool(name="ob", bufs=1) as obp,
        ):
            # warm-up source: small zeroed tile, no DMA dependency
            wz = constp.tile([P, 2, 64], dt.uint8)
            nc.vector.memset(wz[:], 0)

            def hk_src(t_d, kp):
                return t_d[0:P, kp * 1024:(kp + 1) * 1024].rearrange(
                    "p (ko c) -> p ko c", ko=2)

            def kq_src(t_d, kq):
                return t_d[kq * P:(kq + 1) * P, :].rearrange(
                    "p (kpp ko c) -> p kpp ko c", kpp=2, ko=2)

            def b2_src(t_d, i):
                r0 = (KQ + 2 * i) * P
                return t_d[r0:r0 + 2 * P, :].rearrange(
                    "(kq p) (kpp ko c) -> p kq kpp ko c", p=P, kpp=2, ko=2)

            hk = {}   # (op, kp) -> [P, 2, BC]       op 0 = mask, 1 = x
            kqt = {}  # (op, kq) -> [P, 2, 2, BC]    chunk 0, kq 1..7
            b2 = {}   # (op, i)  -> [P, 2, 2, 2, BC] chunk 1, kq 2i..2i+1
            for op, pool in ((0, mkp), (1, xkp)):
                for kp in range(2):
                    hk[(op, kp)] = pool.tile([P, 2, BC], dt.uint8,
                                             name=f"hk{op}_{kp}")
                for kq in range(1, KQ):
                    kqt[(op, kq)] = pool.tile([P, 2, 2, BC], dt.uint8,
                                              name=f"kq{op}_{kq}")
                for i in range(4):
                    b2[(op, i)] = pool.tile([P, 2, 2, 2, BC], dt.uint8,
                                            name=f"b2_{op}_{i}")

            # rate-weighted queue plan (program order per queue matters)
            plan = [
                ('y', hk[(0, 0)], hk_src(m_d, 0)),
                ('s', hk[(1, 0)], hk_src(x_d, 0)),
                ('g', hk[(0, 1)], hk_src(m_d, 1)),
                ('g', hk[(1, 1)], hk_src(x_d, 1)),
                ('y', kqt[(0, 1)], kq_src(m_d, 1)),
                ('s', kqt[(1, 1)], kq_src(x_d, 1)),
                ('g', kqt[(0, 2)], kq_src(m_d, 2)),
                ('g', kqt[(1, 2)], kq_src(x_d, 2)),
                ('y', kqt[(0, 3)], kq_src(m_d, 3)),
                ('s', kqt[(1, 3)], kq_src(x_d, 3)),
                ('g', kqt[(0, 4)], kq_src(m_d, 4)),
                ('y', kqt[(1, 4)], kq_src(x_d, 4)),
                ('s', kqt[(0, 5)], kq_src(m_d, 5)),
                ('g', kqt[(1, 5)], kq_src(x_d, 5)),
                ('g', kqt[(0, 6)], kq_src(m_d, 6)),
                ('y', kqt[(1, 6)], kq_src(x_d, 6)),
                ('s', kqt[(0, 7)], kq_src(m_d, 7)),
                ('g', kqt[(1, 7)], kq_src(x_d, 7)),
                ('s', 'cth', None),
                ('s', 'cng', None),
                ('g', b2[(1, 0)], b2_src(x_d, 0)),
                ('y', b2[(1, 1)], b2_src(x_d, 1)),
                ('s', b2[(1, 2)], b2_src(x_d, 2)),
                ('g', b2[(1, 3)], b2_src(x_d, 3)),
                ('y', b2[(0, 0)], b2_src(m_d, 0)),
                ('s', b2[(0, 1)], b2_src(m_d, 1)),
                ('g', b2[(0, 2)], b2_src(m_d, 2)),
                ('g', b2[(0, 3)], b2_src(m_d, 3)),
            ]
            qmap = {'g': nc.gpsimd, 's': nc.scalar, 'y': nc.sync}
            cth = constp.tile([P, JT], f32)
            cng = constp.tile([P, JT], f32)
            for q, dst, src in plan:
                if dst == 'cth':
                    nc.scalar.dma_start(cth[:], c_d[:])
                elif dst == 'cng':
                    nc.scalar.dma_start(cng[:], cn_d[:])
                else:
                    qmap[q].dma_start(dst[:], src)

            def mm_lhsT(jh, kp, j4):
                jsl = slice(j4 * P, (j4 + 1) * P)
                if jh == 0:
                    if kp < 2:
                        return hk[(0, kp)][:, :, jsl]
                    return kqt[(0, kp // 2)][:, kp % 2, :, jsl]
                kq = kp // 2
                return b2[(0, kq // 2)][:, kq % 2, kp % 2, :, jsl]

            def mm_rhs(bc, kp):
                if bc == 0:
                    if kp < 2:
                        return hk[(1, kp)][:]
                    return kqt[(1, kp // 2)][:, kp % 2, :, :]
                kq = kp // 2
                return b2[(1, kq // 2)][:, kq % 2, kp % 2, :, :]

            # fused output tiles: one per jt-pair [P, 2, BL]
            obs = [obp.tile([P, 2, BL], dt.uint8, name=f"ob{jp}")
                   for jp in range(JT // 2)]

            with tc.tile_pool(name="psacc", bufs=1, space="PSUM") as psacc:
                # PE warm-up: un-throttle HAM while first tiles land
                wps = psacc.tile([P, BC], f32, name="warm", tag="acc1_0")
                for i in range(NWARM):
                    nc.tensor.matmul(
                        wps[0:32, 0:64], wz[:, :, 0:32].bitcast(f8),
                        wz[:].bitcast(f8),
                        start=(i == 0), stop=(i == NWARM - 1), perf_mode=DR)

                passes = [(0, 0), (0, 1), (1, 0), (1, 1)]
                oqs = {0: nc.sync, 1: nc.scalar, 2: nc.scalar, 3: nc.sync}
                for pi, (jh, bc) in enumerate(passes):
                    alt = pi % 2
                    ps = [psacc.tile([P, BC], f32, name=f"acc{pi}_{j4}",
                                     tag=f"acc{alt}_{j4}")
                          for j4 in range(JH)]
                    for kp in range(KP):
                        for j4 in range(JH):
                            nc.tensor.matmul(
                                ps[j4][:],
                                mm_lhsT(jh, kp, j4).bitcast(f8),
                                mm_rhs(bc, kp).bitcast(f8),
                                start=(kp == 0), stop=(kp == KP - 1),
                                perf_mode=DR)
                    for j4 in range(JH):
                        jt = jh * JH + j4
                        osl = obs[jt // 2][:, jt % 2,
                                           bc * BC:(bc + 1) * BC]
                        if j4 % 2:
                            # A' and c are both even, so A'-(c+1) is odd:
                            # Sign never sees 0 and the strict compare is
                            # exact; the uint8 cast saturates -1 to 0.
                            nc.scalar.activation(
                                osl, ps[j4][:], AF.Sign,
                                bias=cng[:, jt:jt + 1], scale=1.0)
                        else:
                            nc.vector.tensor_scalar(
                                osl, ps[j4][:], cth[:, jt:jt + 1], None,
                                op0=ALU.is_gt)
                        if bc == NBC - 1 and j4 % 2:
                            jp = jt // 2
                            dst = o_d[jp * 2 * P:(jp + 1) * 2 * P,
                                      :].rearrange("(j2 p) b -> p j2 b", p=P)
                            oqs[jp].dma_start(dst, obs[jp][:])

    nc.compile()
    return nc


def _get_nc():
    if "nc" not in _cache:
        _cache["nc"] = _build()
    return _cache["nc"]


def _encode_pm1(a01):
    """{0,1} array -> fp8e4 bytes for {-1,+1} (0xB8 / 0x38)."""
    return np.where(a01, np.uint8(0x38), np.uint8(0xB8))


def _tile_k_major(shard):
    """[4096, 1024] byte array (k-major) -> [2*KQ*128, 2048]: row
    (chunk*KQ + kq)*128 + ki = 2048 contiguous bytes covering the two
    k-pairs' ko-interleaved halves of one 512-column chunk."""
    t = shard.reshape(KQ, 2, 2, P, 2, BC)        # [kq, kpp, ko, ki, ch, c]
    t = t.transpose(4, 0, 3, 1, 2, 5)            # [ch, kq, ki, kpp, ko, c]
    return np.ascontiguousarray(t.reshape(2 * KQ * P, 2048))


def run(x, masks, thresholds, trace=False):
    """Run the SPMD kernel on 8 cores. Returns (out_bool, BassKernelResults)."""
    from concourse.bass_utils import run_bass_kernel_spmd

    nc = _get_nc()
    xT8 = np.ascontiguousarray(_encode_pm1(x.T != 0))          # [D, B]
    m8 = _encode_pm1(np.asarray(masks))                        # [D, J]
    cth = (2.0 * thresholds.astype(np.float32) - float(D))     # [J]
    in_maps = []
    for c in range(NCORES):
        bh, jq = c // JS, c % JS
        in_maps.append({
            "xp": _tile_k_major(xT8[:, bh * BL:(bh + 1) * BL]),
            "mp": _tile_k_major(m8[:, jq * JL:(jq + 1) * JL]),
            "cth": np.ascontiguousarray(
                cth[jq * JL:(jq + 1) * JL].reshape(JT, P).T),
            "cng": np.ascontiguousarray(
                -(cth[jq * JL:(jq + 1) * JL] + 1.0).reshape(JT, P).T),
        })
    res = run_bass_kernel_spmd(nc, in_maps, core_ids=list(range(NCORES)),
                               trace=trace)
    out = np.empty((B, J), dtype=np.uint8)
    for c in range(NCORES):
        bh, jq = c // JS, c % JS
        out[bh * BL:(bh + 1) * BL, jq * JL:(jq + 1) * JL] = \
            res.results[c]["out"].T
    return out.view(np.bool_), res


def kernel(x, masks, thresholds):
    x = np.asarray(x)
    masks = np.asarray(masks)
    thresholds = np.asarray(thresholds)
    out, _ = run(x, masks, thresholds, trace=False)
    return out
